# revision 14
# baseline (speedup 1.0000x reference)
"""Trainium2 Bass kernel for ConservativeGSAAttention.

Sharding: 8 cores = 4 batches x 2 head-groups (8 heads each).
Each core computes qkv-proj + attention + its half of c_proj for one batch;
the host sums the two partial c_proj outputs per batch (the "all-reduce").

Layout strategy (per core):
  - hidden_states passed transposed: hsT [E, T].
  - qkv proj computed in transposed layout: qT/kT [feat, token] (feat on
    partitions) so that scores matmuls need no transposes; v computed in
    [token, feat] layout for the AV matmul.
  - scores computed transposed: scoresT [keys, queries] (keys on partitions),
    block [128k x 512q]; fully-masked blocks are skipped; exp on ScalarE with
    the per-head splat scale/bias fused in; triangular 128x128 mask applied
    multiplicatively after exp on the diagonal blocks only.
  - softmax denominator comes for free from an appended ones-column in v
    (AV matmul row 64 = sum_k exp); normalization multiplies the AV output
    (64 rows) instead of the attention matrix (2048 rows).
"""

import math

import numpy as np

import concourse.bass as bass
import concourse.tile as tile
from concourse import bacc
from concourse import mybir
from concourse.bass_utils import run_bass_kernel_spmd

B, T, E, H, D = 4, 2048, 1024, 16, 64
HG = 8              # heads per core
F = HG * D          # 512 feats per group (for each of q, k, v)
P = 128
KT = E // P         # 8 contraction tiles for projections
TT = T // P         # 16 token tiles
QB = 512            # query block width
NQB = T // QB       # 4
FT = F // P         # 4 feat tiles per q/k/v group
FT_QK = 2 * FT      # 8 (q tiles then k tiles)
EB = 512            # c_proj output block width
NEB = E // EB       # 2

f32 = mybir.dt.float32
f32r = mybir.dt.float32r

AF = mybir.ActivationFunctionType


def _r(ap):
    """fp32r view of an fp32 AP for full-rate PE matmuls."""
    return ap.bitcast(f32r)


def build_program():
    nc = bacc.Bacc("TRN2", target_bir_lowering=False, debug=False)

    # ---- I/O ----
    hsT = nc.dram_tensor("hsT", [E, T], f32, kind="ExternalInput").ap()
    wqkT = nc.dram_tensor("wqkT", [E, 2 * F], f32, kind="ExternalInput").ap()
    wvT = nc.dram_tensor("wvT", [E, F], f32, kind="ExternalInput").ap()
    qk_bias = nc.dram_tensor("qk_bias", [P, FT_QK], f32, kind="ExternalInput").ap()
    v_bias = nc.dram_tensor("v_bias", [1, F], f32, kind="ExternalInput").ap()
    wpT = nc.dram_tensor("wpT", [F, E], f32, kind="ExternalInput").ap()
    bp_half = nc.dram_tensor("bp_half", [1, E], f32, kind="ExternalInput").ap()
    tri = nc.dram_tensor("tri", [P, P], f32, kind="ExternalInput").ap()
    act_s = nc.dram_tensor("act_s", [P, HG], f32, kind="ExternalInput").ap()
    act_b = nc.dram_tensor("act_b", [P, HG], f32, kind="ExternalInput").ap()
    out = nc.dram_tensor("out", [TT, P, E], f32, kind="ExternalOutput").ap()

    # ---- DRAM scratch ----
    qT_dr = nc.dram_tensor("qT_dr", [FT, P, T], f32).ap()
    kT_dr = nc.dram_tensor("kT_dr", [FT, P, T], f32).ap()
    v_dr = nc.dram_tensor("v_dr", [HG, TT, P, D + 1], f32).ap()
    ao_dr = nc.dram_tensor("ao_dr", [FT, P, T], f32).ap()
    rc_dr = nc.dram_tensor("rc_dr", [HG, NQB, 1, QB], f32).ap()

    from contextlib import ExitStack
    with tile.TileContext(nc) as tc, ExitStack() as ctx:
        def pool(name, bufs, space="SBUF"):
            return ctx.enter_context(tc.tile_pool(name=name, bufs=bufs, space=space))

        consts = pool("consts", 1)
        big = pool("big", 2)
        hs_pool = pool("hs", 1)
        stage = pool("stage", 3)
        vstage = pool("vstage", 2)
        vh_pool = pool("vh", 2)
        kh_pool = pool("kh", 2)
        q_pool = pool("q", 2)
        ao_pool = pool("aostage", 2)
        bc_pool = pool("bcast", 2)
        rc_pool = pool("rc", 2)
        aol_pool = pool("aol", 4)
        out_pool = pool("outp", 2)
        raw_pool = pool("raw", 3)
        msk_pool = pool("msk", 3)
        mm_ps = pool("mm_ps", 2, "PSUM")
        sc_ps = pool("sc_ps", 2, "PSUM")
        av_ps = pool("av_ps", 2, "PSUM")

        if True:
            # ---- resident constants ----
            wqk_sb = big.tile([P, KT, 2 * F], f32r, tag="big")
            nc.sync.dma_start(out=wqk_sb, in_=wqkT.bitcast(f32r).rearrange("(kt p) f -> p kt f", p=P))
            wv_sb = consts.tile([P, KT, F], f32r)
            nc.sync.dma_start(out=wv_sb, in_=wvT.bitcast(f32r).rearrange("(kt p) f -> p kt f", p=P))
            wp_sb = consts.tile([P, FT, E], f32r)
            nc.sync.dma_start(out=wp_sb, in_=wpT.bitcast(f32r).rearrange("(ft p) e -> p ft e", p=P))
            qkb_sb = consts.tile([P, FT_QK], f32)
            nc.sync.dma_start(out=qkb_sb, in_=qk_bias)
            vb_sb = consts.tile([P, 1, F], f32)
            nc.sync.dma_start(out=vb_sb, in_=v_bias.partition_broadcast(P))
            bp_sb = consts.tile([P, 1, E], f32)
            nc.sync.dma_start(out=bp_sb, in_=bp_half.partition_broadcast(P))
            tri_sb = consts.tile([P, P], f32)
            nc.sync.dma_start(out=tri_sb, in_=tri)
            acts_sb = consts.tile([P, HG], f32)
            nc.sync.dma_start(out=acts_sb, in_=act_s)
            actb_sb = consts.tile([P, HG], f32)
            nc.sync.dma_start(out=actb_sb, in_=act_b)
            ones_col = consts.tile([P, 1], f32)
            nc.vector.memset(ones_col, 1.0)

            hsT_t = hsT.rearrange("(kt p) t -> p kt t", p=P)

            # ---- Phase A: qkv projection (transposed layouts) ----
            for tb in range(NQB):
                hs_t = hs_pool.tile([P, KT, QB], f32r)
                nc.sync.dma_start(out=hs_t, in_=hsT_t[:, :, tb * QB:(tb + 1) * QB].bitcast(f32r))

                # qT / kT : [feat, token]
                for ft in range(FT_QK):
                    ps = mm_ps.tile([P, QB], f32, tag="mm")
                    for kt in range(KT):
                        nc.tensor.matmul(
                            ps,
                            (wqk_sb[:, kt, ft * P:(ft + 1) * P]),
                            (hs_t[:, kt, :]),
                            start=(kt == 0),
                            stop=(kt == KT - 1),
                        )
                    st = stage.tile([P, QB], f32r)
                    nc.scalar.activation(
                        out=st, in_=ps, func=AF.Identity,
                        bias=qkb_sb[:, ft:ft + 1], scale=1.0,
                    )
                    if ft < FT:
                        nc.sync.dma_start(
                            out=qT_dr[ft, :, tb * QB:(tb + 1) * QB].bitcast(f32r), in_=st)
                    else:
                        nc.sync.dma_start(
                            out=kT_dr[ft - FT, :, tb * QB:(tb + 1) * QB].bitcast(f32r), in_=st)

                # v : [token, feat] with ones column appended per head
                for tsub in range(QB // P):
                    tt = tb * (QB // P) + tsub
                    psv = mm_ps.tile([P, F], f32, tag="mm")
                    for kt in range(KT):
                        nc.tensor.matmul(
                            psv,
                            (hs_t[:, kt, tsub * P:(tsub + 1) * P]),
                            (wv_sb[:, kt, :]),
                            start=(kt == 0),
                            stop=(kt == KT - 1),
                        )
                    vt = vstage.tile([P, HG, D + 1], f32r)
                    nc.vector.tensor_add(
                        vt[:, :, 0:D],
                        psv.rearrange("p (h d) -> p h d", h=HG),
                        vb_sb.rearrange("p o (h d) -> p (o h) d", h=HG),
                    )
                    for hh in range(HG):
                        nc.vector.tensor_copy(vt[:, hh, D:D + 1], ones_col)
                    nc.sync.dma_start(
                        out=v_dr[:, tt, :, :].bitcast(f32r).rearrange("h p d -> p h d"), in_=vt)

            # ---- Phase B: attention per head ----
            for h in range(HG):
                hf = h // 2          # feat tile holding this head
                hr = (h % 2) * D     # row offset inside the feat tile
                vh = vh_pool.tile([P, TT, D + 1], f32r)
                nc.sync.dma_start(
                    out=vh, in_=v_dr[h, :, :, :].bitcast(f32r).rearrange("tt p d -> p tt d"))
                kh = kh_pool.tile([D, T], f32r)
                nc.sync.dma_start(out=kh, in_=kT_dr[hf, hr:hr + D, :].bitcast(f32r))

                for qb in range(NQB):
                    nkt = (qb + 1) * (QB // P)
                    qt = q_pool.tile([D, QB], f32r)
                    nc.sync.dma_start(
                        out=qt, in_=qT_dr[hf, hr:hr + D, qb * QB:(qb + 1) * QB].bitcast(f32r))

                    at = big.tile([P, NQB * (QB // P), QB], f32r, tag="big")
                    for kt in range(nkt):
                        ps = sc_ps.tile([P, QB], f32, tag="sc")
                        nc.tensor.matmul(
                            ps,
                            (kh[:, kt * P:(kt + 1) * P]),
                            (qt),
                            start=True, stop=True,
                        )
                        j = kt - qb * (QB // P)  # >=0 on diagonal tiles
                        if j < 0:
                            nc.scalar.activation(
                                out=at[:, kt, :], in_=ps, func=AF.Exp,
                                bias=actb_sb[:, h:h + 1], scale=acts_sb[:, h:h + 1],
                            )
                        else:
                            # Keep `at` ACT-only-written (the AV matmul can
                            # carry just one wait): ACT copies the diagonal
                            # 128 cols to SBUF, DVE adds the -1e30 mask
                            # there, ACT exps it back into `at`.
                            raw = raw_pool.tile([P, P], f32)
                            nc.scalar.activation(
                                out=raw, in_=ps[:, j * P:(j + 1) * P],
                                func=AF.Copy)
                            msk = msk_pool.tile([P, P], f32)
                            nc.vector.tensor_add(msk, raw, tri_sb)
                            if j > 0:
                                nc.scalar.activation(
                                    out=at[:, kt, 0:j * P], in_=ps[:, 0:j * P],
                                    func=AF.Copy, scale=0.0)
                            nc.scalar.activation(
                                out=at[:, kt, j * P:(j + 1) * P], in_=msk,
                                func=AF.Exp,
                                bias=actb_sb[:, h:h + 1], scale=acts_sb[:, h:h + 1],
                            )
                            if j < 3:
                                nc.scalar.activation(
                                    out=at[:, kt, (j + 1) * P:QB],
                                    in_=ps[:, (j + 1) * P:QB],
                                    func=AF.Exp,
                                    bias=actb_sb[:, h:h + 1],
                                    scale=acts_sb[:, h:h + 1],
                                )

                    avp = av_ps.tile([D + 1, QB], f32, tag="av")
                    for kt in range(nkt):
                        nc.tensor.matmul(
                            avp,
                            (vh[:, kt, :]),
                            (at[:, kt, :]),
                            start=(kt == 0),
                            stop=(kt == nkt - 1),
                        )

                    # normalize rows 0..63 by row 64 (the exp-sum)
                    rc = rc_pool.tile([1, QB], f32)
                    nc.vector.reciprocal(rc, avp[D:D + 1, :])
                    nc.sync.dma_start(out=rc_dr[h, qb], in_=rc)
                    bc = bc_pool.tile([D, QB], f32)
                    nc.sync.dma_start(
                        out=bc, in_=rc_dr[h, qb].partition_broadcast(D).rearrange(
                            "p o q -> p (o q)"))
                    ao = ao_pool.tile([D, QB], f32)
                    nc.vector.tensor_mul(ao, avp[0:D, :], bc)
                    nc.sync.dma_start(
                        out=ao_dr[hf, hr:hr + D, qb * QB:(qb + 1) * QB], in_=ao)

            # ---- Phase C: c_proj (partial, + bproj/2) ----
            for tt in range(TT):
                ot = out_pool.tile([P, E], f32)
                for eb in range(NEB):
                    ps = mm_ps.tile([P, EB], f32, tag="mm")
                    for ft in range(FT):
                        lt = aol_pool.tile([P, P], f32r)
                        nc.sync.dma_start(
                            out=lt, in_=ao_dr[ft, :, tt * P:(tt + 1) * P].bitcast(f32r))
                        nc.tensor.matmul(
                            ps,
                            (lt),
                            (wp_sb[:, ft, eb * EB:(eb + 1) * EB]),
                            start=(ft == 0),
                            stop=(ft == FT - 1),
                        )
                    nc.vector.tensor_add(
                        ot[:, eb * EB:(eb + 1) * EB], ps,
                        bp_sb[:, 0, eb * EB:(eb + 1) * EB],
                    )
                nc.sync.dma_start(out=out[tt], in_=ot)

    nc.compile()
    return nc


def make_in_maps(hidden_states, Wqkv, bqkv, Wproj, bproj, splat_scale, splat_bias):
    hs = np.asarray(hidden_states, dtype=np.float32)
    Wqkv = np.asarray(Wqkv, dtype=np.float32)
    bqkv = np.asarray(bqkv, dtype=np.float32)
    Wproj = np.asarray(Wproj, dtype=np.float32)
    bproj = np.asarray(bproj, dtype=np.float32)
    s = (1.0 + 0.01 * np.tanh(np.asarray(splat_scale, dtype=np.float32))).astype(np.float32)
    bsp = (0.001 * np.tanh(np.asarray(splat_bias, dtype=np.float32).reshape(H))).astype(np.float32)
    scale_factor = np.float32(1.0 / math.sqrt(D))

    Wq, Wk, Wv = Wqkv[0:E], Wqkv[E:2 * E], Wqkv[2 * E:3 * E]
    bq, bk, bv = bqkv[0:E], bqkv[E:2 * E], bqkv[2 * E:3 * E]

    tri = np.where(np.arange(P)[None, :] >= np.arange(P)[:, None],
                   np.float32(0.0), np.float32(-1e30)).astype(np.float32)

    group_maps = []
    for g in range(2):
        gs = slice(g * F, (g + 1) * F)
        wqkT = np.ascontiguousarray(
            np.concatenate([Wq[gs], Wk[gs]], axis=0).T).astype(np.float32)
        wvT = np.ascontiguousarray(Wv[gs].T).astype(np.float32)
        qk_bias = np.ascontiguousarray(
            np.concatenate([bq[gs], bk[gs]]).reshape(FT_QK, P).T).astype(np.float32)
        v_bias = np.ascontiguousarray(bv[gs].reshape(1, F)).astype(np.float32)
        wpT = np.ascontiguousarray(Wproj[:, gs].T).astype(np.float32)
        bp = (bproj * 0.5).reshape(1, E).astype(np.float32)
        hsl = slice(g * HG, (g + 1) * HG)
        act_s = np.tile((s[hsl] * scale_factor).reshape(1, HG), (P, 1)).astype(np.float32)
        act_b = np.tile(bsp[hsl].reshape(1, HG), (P, 1)).astype(np.float32)
        group_maps.append(dict(
            wqkT=wqkT, wvT=wvT, qk_bias=qk_bias, v_bias=v_bias,
            wpT=wpT, bp_half=bp, tri=tri, act_s=act_s, act_b=act_b,
        ))

    in_maps = []
    for c in range(8):
        b, g = c // 2, c % 2
        m = dict(group_maps[g])
        m["hsT"] = np.ascontiguousarray(hs[b].T).astype(np.float32)
        in_maps.append(m)
    return in_maps


def kernel(hidden_states, Wqkv, bqkv, Wproj, bproj, splat_scale, splat_bias,
           **run_kwargs):
    in_maps = make_in_maps(hidden_states, Wqkv, bqkv, Wproj, bproj,
                           splat_scale, splat_bias)
    nc = build_program()
    res = run_bass_kernel_spmd(nc, in_maps, core_ids=list(range(8)), **run_kwargs)
    outs = [np.asarray(r["out"], dtype=np.float32).reshape(T, E) for r in res.results]
    full = np.stack([outs[2 * b] + outs[2 * b + 1] for b in range(B)], axis=0)
    return full
